# revision 1
# baseline (speedup 1.0000x reference)
"""Env-specific MLP heads on 8 trn2 cores.

out[i] = Linear2(relu(Linear1(h[i]))) using the weights of head env_ids[i].

Strategy (expert-parallel with host-side routing):
  - Host sorts tokens by env id. Env e's tokens are split between cores
    2e and 2e+1 (E=4 envs, 8 cores), zero-padded to a common length T.
  - Each core runs a dense 2-layer MLP on its [T, D] token block with a
    single env's weights: no masking, no wasted env compute (the
    reference computes all E envs for every token).
  - Activations live in transposed [feature, token] layout on-chip, so
    both matmuls use natural-layout weight tiles as the stationary
    operand and biases are per-partition ACT-engine bias adds. The host
    pre-transposes h (free) and un-permutes the gathered output (free).
"""

import numpy as np
import ml_dtypes

import concourse.mybir as mybir
import concourse.tile as tile
from concourse import bacc
from concourse.bass_utils import run_bass_kernel_spmd

P = 128
NCORES = 8
NMAX = 512  # one fp32 PSUM bank


def build_nc(T, D=1024, Hdim=2048, A=1024, iters=1, compute_only=False, ps_bufs=4, chain_groups=False, nmax=NMAX, seq_codegen=False):
    """Bass program for one core: out[A,T] = W2.T@relu(W1.T@xt + b1) + b2.

    iters>1 repeats the compute phase (for steady-state HW timing only).
    """
    KO1, KO2, AT = D // P, Hdim // P, A // P
    bf16, f32 = mybir.dt.bfloat16, mybir.dt.float32

    nc = bacc.Bacc(
        "TRN2", target_bir_lowering=False, debug=True, num_devices=NCORES,
        use_seq_codegen=seq_codegen,
    )

    xt = nc.dram_tensor("xt", [D, T], bf16, kind="ExternalInput")
    w1 = nc.dram_tensor("w1", [D, Hdim], bf16, kind="ExternalInput")
    b1 = nc.dram_tensor("b1", [P, KO2], f32, kind="ExternalInput")
    w2 = nc.dram_tensor("w2", [Hdim, A], bf16, kind="ExternalInput")
    b2 = nc.dram_tensor("b2", [P, AT], f32, kind="ExternalInput")
    out = nc.dram_tensor("out", [A, T], f32, kind="ExternalOutput")

    # Token axis in PSUM-bank-sized chunks. The final chunk is kept at 128
    # so the kernel tail (last psum group + ACT + out-DMA after the final
    # matmul) is short; N>=128 matmuls are purely streaming-rate-limited so
    # the extra instructions cost no PE time.
    chunks = []
    t0 = 0
    while T - t0 > 0:
        rem = T - t0
        if rem > nmax:
            tn = nmax if rem - nmax >= 256 or rem == 2 * nmax else nmax - 128
        elif rem > 256:
            tn = rem - 128
        else:
            tn = rem
        chunks.append((t0, tn))
        t0 += tn

    with tile.TileContext(nc) as tc:
        with (
            tc.tile_pool(name="weights", bufs=1) as wp,
            tc.tile_pool(name="acts", bufs=1) as acts,
            tc.tile_pool(name="ps1", bufs=ps_bufs, space="PSUM") as pp1,
            tc.tile_pool(name="ps2", bufs=ps_bufs, space="PSUM") as pp2,
            tc.tile_pool(name="outs", bufs=4) as op,
        ):
            w1_sb = wp.tile([P, KO1, Hdim], bf16, tag="w1")
            w2_sb = wp.tile([P, KO2, A], bf16, tag="w2")
            b1_sb = wp.tile([P, KO2], f32, tag="b1")
            b2_sb = wp.tile([P, AT], f32, tag="b2")
            xt_sb = acts.tile([P, KO1, T], bf16, tag="xt")

            # Two HWDGE rings: SP (nc.sync) and ACT (nc.scalar). Alternate
            # w1 k-slices (with the matching first-chunk xt slices) across
            # both rings so the first L1 psum groups can start ASAP; the
            # remaining xt chunks and then w2 stream in behind the L1 phase.
            # DMA plan. The Scalar engine doubles as a HWDGE ring AND runs
            # every activation; a long trigger queue there delays the first
            # relu, which blocks psum recycling and stalls the PE. So the
            # Scalar ring gets ONLY the small earliest batch (half of the
            # w1-low/xt-chunk0 set) and is then free for relus. The SP ring
            # carries the rest of L1's inputs + half of w2; the gpsimd
            # SWDGE queue carries the other half of w2 and all outputs.
            from concourse.tile import add_dep_helper as _dep

            def multi_k(dst3, dram, r0, r1, c0, c1):
                # One DMA covering k-slices r0//P..r1//P: [rows, cols] ->
                # [128, ko, cols] via a 3D access pattern.
                return_ap = dram[r0:r1, c0:c1].rearrange(
                    "(ko p) c -> p ko c", p=P
                )
                return dst3, return_ap

            Hh = Hdim // 2
            t0c0, tnc0 = chunks[0]
            K2 = KO1 // 2
            # Prefix (everything the first 8 L1 groups need) as 4 large
            # DMAs + biases, split across both HWDGE rings so each ring
            # issues only ~2 triggers and the Scalar engine frees quickly.
            nc.scalar.dma_start(*multi_k(w1_sb[:, :K2, :Hh], w1, 0, K2 * P, 0, Hh))
            nc.gpsimd.dma_start(
                *multi_k(xt_sb[:, :, t0c0 : t0c0 + tnc0], xt, 0, KO1 * P, t0c0, t0c0 + tnc0)
            )
            nc.scalar.dma_start(b1_sb[:], b1[:])
            nc.sync.dma_start(b2_sb[:], b2[:])
            last_pre = nc.sync.dma_start(
                *multi_k(w1_sb[:, K2:, :Hh], w1, K2 * P, KO1 * P, 0, Hh)
            )
            # Rest of L1's inputs on the SP ring.
            nc.sync.dma_start(*multi_k(w1_sb[:, :K2, Hh:], w1, 0, K2 * P, Hh, Hdim))
            nc.sync.dma_start(*multi_k(w1_sb[:, K2:, Hh:], w1, K2 * P, KO1 * P, Hh, Hdim))
            for t0, tn in chunks[1:]:
                nc.sync.dma_start(
                    *multi_k(xt_sb[:, :, t0 : t0 + tn], xt, 0, KO1 * P, t0, t0 + tn)
                )
            # w2 in 4 large DMAs: two on the gpsimd SWDGE queue (dep-gated
            # behind the L1 prefix — the SDMA pool is shared and an eager
            # w2 starves the PE's startup inputs), two on the SP ring tail.
            K4 = KO2 // 4
            for j in range(4):
                eng = nc.gpsimd if j % 2 == 0 else nc.sync
                w2_dma = eng.dma_start(
                    *multi_k(w2_sb[:, j * K4 : (j + 1) * K4, :], w2,
                             j * K4 * P, (j + 1) * K4 * P, 0, A)
                )
                if j == 0:
                    _dep(w2_dma.ins, last_pre.ins, sync=True,
                         reason="defer w2 SWDGE behind L1 input prefix")

            # PE pre-warm: the HAM clock-gate starts at K=4/8 (half clock)
            # and needs ~3.4us of sustained PE activity to unthrottle. Real
            # work can't start until the input prefix lands (~13-18us), so
            # without this the first ~8us of real matmuls run at half rate.
            # Run dummy matmuls on memset scratch during the DMA wait so
            # the PE is warm when real inputs arrive (scratch psum shares
            # the ps2 slots; it is written, never read).
            wsc = wp.tile([P, P], bf16, tag="warm_w")
            xsc = wp.tile([P, NMAX], bf16, tag="warm_x")
            nc.vector.memset(wsc[:], 0.0)
            nc.vector.memset(xsc[:], 0.0)
            ps_warm = pp2.tile([P, NMAX], f32, tag="ps2", name="ps_warm")
            for _ in range(28):
                nc.tensor.matmul(ps_warm[:], wsc[:], xsc[:], start=True, stop=True)

            from concourse.tile import add_dep_helper

            def emit_compute():
                # Optionally chain psum groups so the PE stream runs strictly
                # group-sequential (same psum bank for consecutive matmuls)
                # instead of the scheduler's interleaved order.
                prev_last = [None]

                def mm(*args, first=False, **kw):
                    ins = nc.tensor.matmul(*args, **kw)
                    if chain_groups:
                        if first and prev_last[0] is not None:
                            add_dep_helper(ins.ins, prev_last[0], sync=False,
                                           reason="group chain")
                        prev_last[0] = ins.ins
                    return ins

                hid_tiles = {}
                for t0, tn in chunks:
                    # hid^T[H, t0:t0+tn] as KO2 tiles of [128 features, tn]
                    hid_sb = acts.tile([P, KO2, tn], bf16, tag=f"hid_{t0}", name=f"hid_{t0}")
                    hid_tiles[t0] = hid_sb
                first_block = [True]
                for hhalf in range(2):
                  for t0, tn in chunks:
                    hid_sb = hid_tiles[t0]
                    h_lo = hhalf * KO2 // 2
                    h_hi = (hhalf + 1) * KO2 // 2
                    if first_block[0]:
                        # Split-K interleave for the first 4 psum groups:
                        # all their k<KO1/2 matmuls run first (those weight
                        # slices arrive on the fast scalar-ring prefix), so
                        # the PE has ~3.5us of work in flight while the
                        # second w1-low half is still in transit.
                        first_block[0] = False
                        W = min(4, h_hi - h_lo)
                        pss = [
                            pp1.tile([P, tn], f32, tag="ps1", name="ps1")
                            for _ in range(W)
                        ]
                        Kh = KO1 // 2
                        for kpass in (range(Kh), range(Kh, KO1)):
                            for hi in range(W):
                                h = h_lo + hi
                                for k in kpass:
                                    mm(
                                        pss[hi][:],
                                        w1_sb[:, k, h * P : (h + 1) * P],
                                        xt_sb[:, k, t0 : t0 + tn],
                                        start=(k == 0),
                                        stop=(k == KO1 - 1),
                                        first=(k == 0),
                                    )
                        for hi in range(W):
                            h = h_lo + hi
                            if not compute_only:
                                nc.scalar.activation(
                                    hid_sb[:, h],
                                    pss[hi][:],
                                    mybir.ActivationFunctionType.Relu,
                                    bias=b1_sb[:, h : h + 1],
                                )
                        h_start = h_lo + W
                    else:
                        h_start = h_lo
                    for h in range(h_start, h_hi):
                        ps = pp1.tile([P, tn], f32, tag="ps1", name="ps1")
                        for ki, k in enumerate(range(KO1)):
                            mm(
                                ps[:],
                                w1_sb[:, k, h * P : (h + 1) * P],
                                xt_sb[:, k, t0 : t0 + tn],
                                start=(ki == 0),
                                stop=(ki == KO1 - 1),
                                first=(ki == 0),
                            )
                        if not compute_only:
                            nc.scalar.activation(
                                hid_sb[:, h],
                                ps[:],
                                mybir.ActivationFunctionType.Relu,
                                bias=b1_sb[:, h : h + 1],
                            )
                for t0, tn in chunks:
                    hid_sb = hid_tiles[t0]
                    for a in range(AT):
                        ps = pp2.tile([P, tn], f32, tag="ps2", name="ps2")
                        for k in range(KO2):
                            rhs = (
                                hid_sb[:, k]
                                if not compute_only
                                else xt_sb[:, k % KO1, t0 : t0 + tn]
                            )
                            mm(
                                ps[:],
                                w2_sb[:, k, a * P : (a + 1) * P],
                                rhs,
                                start=(k == 0),
                                stop=(k == KO2 - 1),
                                first=(k == 0),
                            )
                        if not compute_only:
                            ot = op.tile([P, tn], f32, tag="ot", name="ot")
                            nc.scalar.activation(
                                ot[:],
                                ps[:],
                                mybir.ActivationFunctionType.Identity,
                                bias=b2_sb[:, a : a + 1],
                            )
                            out_eng = (
                                nc.sync if (t0, tn) == chunks[-1] else nc.gpsimd
                            )
                            out_eng.dma_start(
                                out[a * P : (a + 1) * P, t0 : t0 + tn], ot[:]
                            )

            for _ in range(iters):
                emit_compute()

    nc.compile()
    return nc


def make_in_maps(h, env_ids, W1, b1, W2, b2):
    """Route tokens to cores.

    T is fixed at 1024 so the device kernel is two clean 512-wide chunks
    with no inefficient remainder matmuls. Each env gets 2 cores (2048
    token capacity); the few tokens beyond that for over-represented envs
    go to `overflow` and are computed on the host in fp32.

    Returns (in_maps, core_tokens, overflow, T).
    """
    bf16 = ml_dtypes.bfloat16
    B, D = h.shape
    E, _, Hdim = W1.shape
    A = W2.shape[-1]
    cpe = NCORES // E  # cores per env
    assert cpe * E == NCORES
    T = 1024

    env = np.asarray(env_ids).reshape(-1).astype(np.int64)
    order = np.argsort(env, kind="stable")
    counts = np.bincount(env, minlength=E)
    starts = np.concatenate([[0], np.cumsum(counts)])

    in_maps = []
    core_tokens = []
    overflow = []  # (env, token index array)
    for e in range(E):
        idx = order[starts[e] : starts[e + 1]]
        if len(idx) > cpe * T:
            overflow.append((e, idx[cpe * T :]))
            idx = idx[: cpe * T]
        parts = np.array_split(idx, cpe)
        w1e = np.ascontiguousarray(W1[e]).astype(bf16)
        w2e = np.ascontiguousarray(W2[e]).astype(bf16)
        b1e = np.ascontiguousarray(
            b1[e].astype(np.float32).reshape(Hdim // P, P).T
        )
        b2e = np.ascontiguousarray(b2[e].astype(np.float32).reshape(A // P, P).T)
        for s in range(cpe):
            tok = parts[s]
            xt = np.zeros((D, T), dtype=bf16)
            if len(tok):
                xt[:, : len(tok)] = h[tok].astype(bf16).T
            in_maps.append({"xt": xt, "w1": w1e, "b1": b1e, "w2": w2e, "b2": b2e})
            core_tokens.append(tok)
    return in_maps, core_tokens, overflow, T


def kernel(h, env_ids, W1, b1, W2, b2):
    h = np.asarray(h, dtype=np.float32)
    W1 = np.asarray(W1, dtype=np.float32)
    b1 = np.asarray(b1, dtype=np.float32)
    W2 = np.asarray(W2, dtype=np.float32)
    b2 = np.asarray(b2, dtype=np.float32)

    in_maps, core_tokens, overflow, T = make_in_maps(h, env_ids, W1, b1, W2, b2)
    nc = build_nc(T, D=h.shape[1], Hdim=W1.shape[2], A=W2.shape[2])
    res = run_bass_kernel_spmd(nc, in_maps, list(range(NCORES))).results

    B = h.shape[0]
    A = W2.shape[2]
    out = np.zeros((B, A), dtype=np.float32)
    for c in range(NCORES):
        tok = core_tokens[c]
        if len(tok):
            out[tok] = res[c]["out"][:, : len(tok)].T
    for e, tok in overflow:
        hid = np.maximum(h[tok] @ W1[e] + b1[e], 0.0)
        out[tok] = hid @ W2[e] + b2[e]
    return out



# revision 11
# speedup vs baseline: 1.3263x; 1.3263x over previous
"""Env-specific MLP heads on 8 trn2 cores.

out[i] = Linear2(relu(Linear1(h[i]))) using the weights of head env_ids[i].

Strategy (expert-parallel with host-side routing):
  - Host sorts tokens by env id. Env e's tokens are split between cores
    2e and 2e+1 (E=4 envs, 8 cores), zero-padded to a common length T.
  - Each core runs a dense 2-layer MLP on its [T, D] token block with a
    single env's weights: no masking, no wasted env compute (the
    reference computes all E envs for every token).
  - Activations live in transposed [feature, token] layout on-chip, so
    both matmuls use natural-layout weight tiles as the stationary
    operand and biases are per-partition ACT-engine bias adds. The host
    pre-transposes h (free) and un-permutes the gathered output (free).
"""

import numpy as np
import ml_dtypes

import concourse.mybir as mybir
import concourse.tile as tile
from concourse import bacc
from concourse.bass_utils import run_bass_kernel_spmd

P = 128
NCORES = 8
NMAX = 512  # one fp32 PSUM bank


def _dedup_ldweights(nc):
    """Delete back-to-back InstLdweights with identical weight APs.

    The PE keeps the stationary operand loaded across matmuls; when two
    consecutive PE matmuls use the same stationary tile, the second
    Ldweights (~107ns serialized on HW) is redundant. Only waitless,
    updateless Ldweights are deleted; any other PE instruction resets
    the tracked state.
    """
    removed = 0
    for blk in nc.m.functions[0].blocks:
        last_key = None
        keep = []
        n_dead = 0
        for x in blk.instructions:
            eng = getattr(x, "engine", None)
            if eng == mybir.EngineType.PE:
                if isinstance(x, mybir.InstLdweights):
                    si = x.sync_info
                    clean = si is None or (
                        len(si.on_wait) == 0 and len(si.on_update) == 0
                    )
                    key = (
                        str(x.ins[0]),
                        str(x.tile_size),
                        str(x.tile_position),
                        str(x.perf_mode),
                        str(x.is_transpose),
                    )
                    if clean and key == last_key:
                        n_dead += 1
                        continue
                    last_key = key
                elif not isinstance(x, mybir.InstMatmult):
                    last_key = None
            keep.append(x)
        if n_dead:
            blk.instructions = keep
        removed += n_dead
    return removed


def build_nc(T, D=1024, Hdim=2048, A=1024, iters=1, compute_only=False, ps_bufs=4, chain_groups=False, nmax=NMAX, seq_codegen=False, pair=True, chunks_override=None, dedup_ldw=None):
    """Bass program for one core: out[A,T] = W2.T@relu(W1.T@xt + b1) + b2.

    iters>1 repeats the compute phase (for steady-state HW timing only).
    pair=True reorders the matmul loops so both token chunks stream
    against each stationary weight tile back-to-back (PE program order
    pinned via nosync chaining) and then drops the redundant second
    Ldweights of each pair post-compile.
    """
    if dedup_ldw is None:
        dedup_ldw = pair
    KO1, KO2, AT = D // P, Hdim // P, A // P
    bf16, f32 = mybir.dt.bfloat16, mybir.dt.float32

    nc = bacc.Bacc(
        "TRN2", target_bir_lowering=False, debug=True, num_devices=NCORES,
        use_seq_codegen=seq_codegen,
    )

    xt = nc.dram_tensor("xt", [D, T], bf16, kind="ExternalInput")
    w1 = nc.dram_tensor("w1", [D, Hdim], bf16, kind="ExternalInput")
    b1 = nc.dram_tensor("b1", [P, KO2], f32, kind="ExternalInput")
    w2 = nc.dram_tensor("w2", [Hdim, A], bf16, kind="ExternalInput")
    b2 = nc.dram_tensor("b2", [P, AT], f32, kind="ExternalInput")
    out = nc.dram_tensor("out", [A, T], f32, kind="ExternalOutput")

    # Token axis in PSUM-bank-sized chunks. The final chunk is kept at 128
    # so the kernel tail (last psum group + ACT + out-DMA after the final
    # matmul) is short; N>=128 matmuls are purely streaming-rate-limited so
    # the extra instructions cost no PE time.
    if chunks_override is not None:
        chunks = list(chunks_override)
    elif pair and T % 2 == 0 and T // 2 <= nmax:
        # Two equal PSUM-bank-sized chunks: every stationary weight tile
        # feeds exactly one matmul per chunk, back-to-back.
        chunks = [(0, T // 2), (T // 2, T // 2)]
    else:
        chunks = []
        t0 = 0
        while T - t0 > 0:
            rem = T - t0
            if rem > nmax:
                tn = nmax if rem - nmax >= 256 or rem == 2 * nmax else nmax - 128
            elif rem > 256:
                tn = rem - 128
            else:
                tn = rem
            chunks.append((t0, tn))
            t0 += tn

    with tile.TileContext(nc) as tc:
        with (
            tc.tile_pool(name="weights", bufs=1) as wp,
            tc.tile_pool(name="acts", bufs=1) as acts,
            tc.tile_pool(name="ps1", bufs=ps_bufs, space="PSUM") as pp1,
            tc.tile_pool(name="ps2", bufs=ps_bufs, space="PSUM") as pp2,
            tc.tile_pool(name="outs", bufs=4) as op,
        ):
            w1_sb = wp.tile([P, KO1, Hdim], bf16, tag="w1")
            w2_sb = wp.tile([P, KO2, A], bf16, tag="w2")
            b1_sb = wp.tile([P, KO2], f32, tag="b1")
            b2_sb = wp.tile([P, AT], f32, tag="b2")
            xt_sb = acts.tile([P, KO1, T], bf16, tag="xt")

            # Two HWDGE rings: SP (nc.sync) and ACT (nc.scalar). Alternate
            # w1 k-slices (with the matching first-chunk xt slices) across
            # both rings so the first L1 psum groups can start ASAP; the
            # remaining xt chunks and then w2 stream in behind the L1 phase.
            # DMA plan. The Scalar engine doubles as a HWDGE ring AND runs
            # every activation; a long trigger queue there delays the first
            # relu, which blocks psum recycling and stalls the PE. So the
            # Scalar ring gets ONLY the small earliest batch (half of the
            # w1-low/xt-chunk0 set) and is then free for relus. The SP ring
            # carries the rest of L1's inputs + half of w2; the gpsimd
            # SWDGE queue carries the other half of w2 and all outputs.
            from concourse.tile import add_dep_helper as _dep

            def multi_k(dst3, dram, r0, r1, c0, c1):
                # One DMA covering k-slices r0//P..r1//P: [rows, cols] ->
                # [128, ko, cols] via a 3D access pattern.
                return_ap = dram[r0:r1, c0:c1].rearrange(
                    "(ko p) c -> p ko c", p=P
                )
                return dst3, return_ap

            Hh = Hdim // 2
            t0c0, tnc0 = chunks[0]
            K2 = KO1 // 2
            # Prefix (everything the first 8 L1 groups need) as 4 large
            # DMAs + biases, split across both HWDGE rings so each ring
            # issues only ~2 triggers and the Scalar engine frees quickly.
            nc.scalar.dma_start(*multi_k(w1_sb[:, :K2, :Hh], w1, 0, K2 * P, 0, Hh))
            nc.gpsimd.dma_start(
                *multi_k(xt_sb[:, :, t0c0 : t0c0 + tnc0], xt, 0, KO1 * P, t0c0, t0c0 + tnc0)
            )
            nc.scalar.dma_start(b1_sb[:], b1[:])
            nc.sync.dma_start(b2_sb[:], b2[:])
            last_pre = nc.sync.dma_start(
                *multi_k(w1_sb[:, K2:, :Hh], w1, K2 * P, KO1 * P, 0, Hh)
            )
            # Rest of L1's inputs on the SP ring.
            nc.sync.dma_start(*multi_k(w1_sb[:, :K2, Hh:], w1, 0, K2 * P, Hh, Hdim))
            nc.sync.dma_start(*multi_k(w1_sb[:, K2:, Hh:], w1, K2 * P, KO1 * P, Hh, Hdim))
            for t0, tn in chunks[1:]:
                # In pair mode the very first psum group streams BOTH
                # chunks, so the second xt chunk rides the scalar ring
                # (right behind the small w1 prefix) to land in parallel
                # with the sync ring's w1 pieces.
                xt_eng = nc.scalar if pair else nc.sync
                xt_eng.dma_start(
                    *multi_k(xt_sb[:, :, t0 : t0 + tn], xt, 0, KO1 * P, t0, t0 + tn)
                )
            # w2 in 4 large DMAs: two on the gpsimd SWDGE queue (dep-gated
            # behind the L1 prefix — the SDMA pool is shared and an eager
            # w2 starves the PE's startup inputs), two on the SP ring tail.
            K4 = KO2 // 4
            for j in range(4):
                eng = nc.gpsimd if j % 2 == 0 else nc.sync
                w2_dma = eng.dma_start(
                    *multi_k(w2_sb[:, j * K4 : (j + 1) * K4, :], w2,
                             j * K4 * P, (j + 1) * K4 * P, 0, A)
                )
                if j == 0:
                    _dep(w2_dma.ins, last_pre.ins, sync=True,
                         reason="defer w2 SWDGE behind L1 input prefix")

            # PE pre-warm: the HAM clock-gate starts at K=4/8 (half clock)
            # and needs ~3.4us of sustained PE activity to unthrottle. Real
            # work can't start until the input prefix lands (~13-18us), so
            # without this the first ~8us of real matmuls run at half rate.
            # Run dummy matmuls on memset scratch during the DMA wait so
            # the PE is warm when real inputs arrive (scratch psum shares
            # the ps2 slots; it is written, never read).
            wsc = wp.tile([P, P], bf16, tag="warm_w")
            xsc = wp.tile([P, NMAX], bf16, tag="warm_x")
            nc.vector.memset(wsc[:], 0.0)
            nc.vector.memset(xsc[:], 0.0)
            ps_warm = pp2.tile([P, NMAX], f32, tag="ps2", name="ps_warm")
            for _ in range(28):
                nc.tensor.matmul(ps_warm[:], wsc[:], xsc[:], start=True, stop=True)

            from concourse.tile import add_dep_helper

            def emit_compute():
                # Optionally chain psum groups so the PE stream runs strictly
                # group-sequential (same psum bank for consecutive matmuls)
                # instead of the scheduler's interleaved order.
                prev_last = [None]

                def mm(*args, first=False, **kw):
                    ins = nc.tensor.matmul(*args, **kw)
                    if pair:
                        # Pin total PE order to emission order so the
                        # same-stationary pairs stay adjacent for the
                        # post-compile Ldweights dedup.
                        if prev_last[0] is not None:
                            add_dep_helper(ins.ins, prev_last[0], sync=False,
                                           reason="mm chain")
                        prev_last[0] = ins.ins
                    elif chain_groups:
                        if first and prev_last[0] is not None:
                            add_dep_helper(ins.ins, prev_last[0], sync=False,
                                           reason="group chain")
                        prev_last[0] = ins.ins
                    return ins

                if pair:
                    emit_compute_pair(mm)
                    return

                hid_tiles = {}
                for t0, tn in chunks:
                    # hid^T[H, t0:t0+tn] as KO2 tiles of [128 features, tn]
                    hid_sb = acts.tile([P, KO2, tn], bf16, tag=f"hid_{t0}", name=f"hid_{t0}")
                    hid_tiles[t0] = hid_sb
                first_block = [True]
                for hhalf in range(2):
                  for t0, tn in chunks:
                    hid_sb = hid_tiles[t0]
                    h_lo = hhalf * KO2 // 2
                    h_hi = (hhalf + 1) * KO2 // 2
                    if first_block[0]:
                        # Split-K interleave for the first 4 psum groups:
                        # all their k<KO1/2 matmuls run first (those weight
                        # slices arrive on the fast scalar-ring prefix), so
                        # the PE has ~3.5us of work in flight while the
                        # second w1-low half is still in transit.
                        first_block[0] = False
                        W = min(4, h_hi - h_lo)
                        pss = [
                            pp1.tile([P, tn], f32, tag="ps1", name="ps1")
                            for _ in range(W)
                        ]
                        Kh = KO1 // 2
                        for kpass in (range(Kh), range(Kh, KO1)):
                            for hi in range(W):
                                h = h_lo + hi
                                for k in kpass:
                                    mm(
                                        pss[hi][:],
                                        w1_sb[:, k, h * P : (h + 1) * P],
                                        xt_sb[:, k, t0 : t0 + tn],
                                        start=(k == 0),
                                        stop=(k == KO1 - 1),
                                        first=(k == 0),
                                    )
                        for hi in range(W):
                            h = h_lo + hi
                            if not compute_only:
                                nc.scalar.activation(
                                    hid_sb[:, h],
                                    pss[hi][:],
                                    mybir.ActivationFunctionType.Relu,
                                    bias=b1_sb[:, h : h + 1],
                                )
                        h_start = h_lo + W
                    else:
                        h_start = h_lo
                    for h in range(h_start, h_hi):
                        ps = pp1.tile([P, tn], f32, tag="ps1", name="ps1")
                        for ki, k in enumerate(range(KO1)):
                            mm(
                                ps[:],
                                w1_sb[:, k, h * P : (h + 1) * P],
                                xt_sb[:, k, t0 : t0 + tn],
                                start=(ki == 0),
                                stop=(ki == KO1 - 1),
                                first=(ki == 0),
                            )
                        if not compute_only:
                            nc.scalar.activation(
                                hid_sb[:, h],
                                ps[:],
                                mybir.ActivationFunctionType.Relu,
                                bias=b1_sb[:, h : h + 1],
                            )
                for t0, tn in chunks:
                    hid_sb = hid_tiles[t0]
                    for a in range(AT):
                        ps = pp2.tile([P, tn], f32, tag="ps2", name="ps2")
                        for k in range(KO2):
                            rhs = (
                                hid_sb[:, k]
                                if not compute_only
                                else xt_sb[:, k % KO1, t0 : t0 + tn]
                            )
                            mm(
                                ps[:],
                                w2_sb[:, k, a * P : (a + 1) * P],
                                rhs,
                                start=(k == 0),
                                stop=(k == KO2 - 1),
                                first=(k == 0),
                            )
                        if not compute_only:
                            ot = op.tile([P, tn], f32, tag="ot", name="ot")
                            nc.scalar.activation(
                                ot[:],
                                ps[:],
                                mybir.ActivationFunctionType.Identity,
                                bias=b2_sb[:, a : a + 1],
                            )
                            out_eng = (
                                nc.sync if (t0, tn) == chunks[-1] else nc.gpsimd
                            )
                            out_eng.dma_start(
                                out[a * P : (a + 1) * P, t0 : t0 + tn], ot[:]
                            )

            def emit_compute_pair(mm):
                # Each stationary weight tile feeds one matmul per token
                # chunk, back-to-back, so the second+ Ldweights of the
                # group dedups away. Two psum accumulation groups (one
                # per chunk) are open at a time.
                hid_tiles = {
                    t0: acts.tile(
                        [P, KO2, tn], bf16, tag=f"hid_{t0}", name=f"hid_{t0}"
                    )
                    for t0, tn in chunks
                }
                for h in range(KO2):
                    pss = [
                        pp1.tile([P, tn], f32, tag="ps1", name="ps1")
                        for _, tn in chunks
                    ]
                    for k in range(KO1):
                        for ci, (t0, tn) in enumerate(chunks):
                            mm(
                                pss[ci][:],
                                w1_sb[:, k, h * P : (h + 1) * P],
                                xt_sb[:, k, t0 : t0 + tn],
                                start=(k == 0),
                                stop=(k == KO1 - 1),
                            )
                    if not compute_only:
                        for ci, (t0, tn) in enumerate(chunks):
                            nc.scalar.activation(
                                hid_tiles[t0][:, h],
                                pss[ci][:],
                                mybir.ActivationFunctionType.Relu,
                                bias=b1_sb[:, h : h + 1],
                            )
                for a in range(AT):
                    pss = [
                        pp2.tile([P, tn], f32, tag="ps2", name="ps2")
                        for _, tn in chunks
                    ]
                    for k in range(KO2):
                        for ci, (t0, tn) in enumerate(chunks):
                            rhs = (
                                hid_tiles[t0][:, k]
                                if not compute_only
                                else xt_sb[:, k % KO1, t0 : t0 + tn]
                            )
                            mm(
                                pss[ci][:],
                                w2_sb[:, k, a * P : (a + 1) * P],
                                rhs,
                                start=(k == 0),
                                stop=(k == KO2 - 1),
                            )
                    if not compute_only:
                        for ci, (t0, tn) in enumerate(chunks):
                            ot = op.tile([P, tn], f32, tag="ot", name="ot")
                            nc.scalar.activation(
                                ot[:],
                                pss[ci][:],
                                mybir.ActivationFunctionType.Identity,
                                bias=b2_sb[:, a : a + 1],
                            )
                            out_eng = (
                                nc.sync if (t0, tn) == chunks[-1] else nc.gpsimd
                            )
                            out_eng.dma_start(
                                out[a * P : (a + 1) * P, t0 : t0 + tn], ot[:]
                            )

            for _ in range(iters):
                emit_compute()

    nc.compile()
    if dedup_ldw:
        _dedup_ldweights(nc)
    return nc


def make_in_maps(h, env_ids, W1, b1, W2, b2):
    """Route tokens to cores.

    T is fixed at 1024 so the device kernel is two clean 512-wide chunks
    with no inefficient remainder matmuls. Each env gets 2 cores (2048
    token capacity); the few tokens beyond that for over-represented envs
    go to `overflow` and are computed on the host in fp32.

    Returns (in_maps, core_tokens, overflow, T).
    """
    bf16 = ml_dtypes.bfloat16
    B, D = h.shape
    E, _, Hdim = W1.shape
    A = W2.shape[-1]
    cpe = NCORES // E  # cores per env
    assert cpe * E == NCORES
    T = 1024

    env = np.asarray(env_ids).reshape(-1).astype(np.int64)
    order = np.argsort(env, kind="stable")
    counts = np.bincount(env, minlength=E)
    starts = np.concatenate([[0], np.cumsum(counts)])

    in_maps = []
    core_tokens = []
    overflow = []  # (env, token index array)
    for e in range(E):
        idx = order[starts[e] : starts[e + 1]]
        if len(idx) > cpe * T:
            overflow.append((e, idx[cpe * T :]))
            idx = idx[: cpe * T]
        parts = np.array_split(idx, cpe)
        w1e = np.ascontiguousarray(W1[e]).astype(bf16)
        w2e = np.ascontiguousarray(W2[e]).astype(bf16)
        b1e = np.ascontiguousarray(
            b1[e].astype(np.float32).reshape(Hdim // P, P).T
        )
        b2e = np.ascontiguousarray(b2[e].astype(np.float32).reshape(A // P, P).T)
        for s in range(cpe):
            tok = parts[s]
            xt = np.zeros((D, T), dtype=bf16)
            if len(tok):
                xt[:, : len(tok)] = h[tok].astype(bf16).T
            in_maps.append({"xt": xt, "w1": w1e, "b1": b1e, "w2": w2e, "b2": b2e})
            core_tokens.append(tok)
    return in_maps, core_tokens, overflow, T


def kernel(h, env_ids, W1, b1, W2, b2):
    h = np.asarray(h, dtype=np.float32)
    W1 = np.asarray(W1, dtype=np.float32)
    b1 = np.asarray(b1, dtype=np.float32)
    W2 = np.asarray(W2, dtype=np.float32)
    b2 = np.asarray(b2, dtype=np.float32)

    in_maps, core_tokens, overflow, T = make_in_maps(h, env_ids, W1, b1, W2, b2)
    nc = build_nc(T, D=h.shape[1], Hdim=W1.shape[2], A=W2.shape[2])
    res = run_bass_kernel_spmd(nc, in_maps, list(range(NCORES))).results

    B = h.shape[0]
    A = W2.shape[2]
    out = np.zeros((B, A), dtype=np.float32)
    for c in range(NCORES):
        tok = core_tokens[c]
        if len(tok):
            out[tok] = res[c]["out"][:, : len(tok)].T
    for e, tok in overflow:
        hid = np.maximum(h[tok] @ W1[e] + b1[e], 0.0)
        out[tok] = hid @ W2[e] + b2[e]
    return out

